# revision 18
# baseline (speedup 1.0000x reference)
"""MSRSA multi-head attention kernel for 8 Trainium2 NeuronCores.

Strategy: data-parallel over batch (B=8 -> 1 batch element per core).
Per core, for its batch element b:
  Qt = (W_q/8) @ queries^T        [512,1024]  (scale 1/8 folded into W_q)
  Kt = W_k @ keys^T               [512,1024]
  V  = values @ W_v^T             [1024,512]  (rows masked by attention_mask)
  per head h, scores are computed TRANSPOSED: S_T[k,q]:
     S_T = sum_d Kt[d,k]*Qt[d,q] + la[h]*A^T[k,q] + ld[h]*D^T[k,q]
  (A/D bias injected by scaled-identity matmuls accumulating into PSUM)
  expS = exp(S_T) on ScalarE (PSUM -> SBUF evacuation is the exp)
  attnT_h[d,q] (+ denominator row) = sum_k V_ext[k, d|mask] * expS[k,q]
  (mask column of V_ext -> row 64 of PV output = softmax denominator)
  normalize: denom row copied to partition 0 (copies may retarget the output
  base), reciprocal_approx_fast at base 0, fp16 cast, K=1 ones-matmul
  broadcast; the multiply writes even heads at partitions 0:64 and odd heads
  at 64:128 directly.
  out = attnT contracted with W_o^T   [1024, 512] (fp16, host upcasts)

On TRN2 a matmul costs ~N output columns at 1 col/cycle regardless of dtype
or contraction size, so the kernel minimizes matmul COUNT and keeps weight
loads small (fp16 64-col ident tiles) so LDWEIGHTS hides under execution.
"""

import contextlib

import numpy as np

import concourse.bass as bass
import concourse.mybir as mybir
import concourse.tile as tile
from concourse.bass_utils import run_bass_kernel_spmd

B, L, DIN, DM, H = 8, 1024, 256, 512, 8
DH = DM // H  # 64
P = 128
NKT = L // P          # 8 k-tiles
NQC = 2               # q chunks
QC = L // NQC         # 512
F32 = mybir.dt.float32
F16 = mybir.dt.float16


def _emit(tc):
    nc = tc.nc

    def dram(name, shape, dtype=F16, kind="ExternalInput"):
        return nc.dram_tensor(name, shape, dtype, kind=kind).ap()

    qT = dram("qT", [DIN, L])
    kT = dram("kT", [DIN, L])
    vT = dram("vT", [DIN, L])
    wqT = dram("wqT", [DIN, DM])
    wkT = dram("wkT", [DIN, DM])
    wvT = dram("wvT", [DIN, DM])
    woT = dram("woT", [DM, DM])
    adT = dram("adT", [2 * L, L])  # A^T/D^T interleaved in 64-row blocks
    identsc = dram("identsc", [P, H * DH])
    mask01 = dram("mask01", [P, NKT], F32)
    out = dram("out", [L, DM], F16, kind="ExternalOutput")

    with contextlib.ExitStack() as ctx:
        singles = ctx.enter_context(tc.tile_pool(name="singles", bufs=1))
        big = ctx.enter_context(tc.tile_pool(name="big", bufs=1))
        exps = ctx.enter_context(tc.tile_pool(name="exps", bufs=3))
        small = ctx.enter_context(tc.tile_pool(name="small", bufs=2))
        spsum = ctx.enter_context(tc.tile_pool(name="spsum", bufs=2, space="PSUM"))
        pvwo = ctx.enter_context(tc.tile_pool(name="pvwo", bufs=3, space="PSUM"))
        bcp = ctx.enter_context(tc.tile_pool(name="bcp", bufs=1, space="PSUM"))

        # ---- big SBUF-resident tensors ----
        ad_sb = big.tile([P, 2 * NKT, L], F16, tag="ad")  # A^T|D^T 64-row blocks
        qt_sb = big.tile([P, 4, L], F16, tag="qt")       # [p,t,l] = Qt[t*128+p, l]
        kt_sb = big.tile([P, 4, L], F16, tag="kt")
        vx_sb = big.tile([P, NKT, H, DH + 1], F16, tag="vx")  # V + mask column
        attnT_sb = big.tile([P, 4, QC], F16, tag="attnT")     # per q-chunk

        adT_r = adT.rearrange("(t p) q -> p t q", p=P)

        # ---- phase 1: projections (pools scoped so SBUF is reclaimed) ----
        proj_ctx = contextlib.ExitStack()
        stage = proj_ctx.enter_context(tc.tile_pool(name="stage", bufs=3))
        wpool = proj_ctx.enter_context(tc.tile_pool(name="wpool", bufs=3))

        def load_stage(src, eng):
            t = stage.tile([P, 2, L], F16, tag="stage")
            eng.dma_start(out=t[:], in_=src.rearrange("(t p) l -> p t l", p=P))
            return t

        def load_w(src, eng):
            t = wpool.tile([P, 2, DM], F16, tag="w")
            eng.dma_start(out=t[:], in_=src.rearrange("(t p) d -> p t d", p=P))
            return t

        # DMA issue order = dependency order: Q/K paths gate the first
        # matmuls, idents+ad gate the first bias matmul, V/Wo come later.
        # Issue across both HWDGE engines (sync + scalar) so descriptor
        # generation is not serialized at the head of the kernel.
        q_sb, wq_sb = load_stage(qT, nc.sync), load_w(wqT, nc.scalar)
        k_sb, wk_sb = load_stage(kT, nc.sync), load_w(wkT, nc.scalar)

        idents = singles.tile([P, H, DH], F16, tag="idents")
        nc.scalar.dma_start(
            out=idents[:], in_=identsc.rearrange("p (j m) -> p j m", m=DH)
        )
        for t in range(2 * NKT):
            eng = nc.sync if t % 2 == 0 else nc.scalar
            eng.dma_start(out=ad_sb[:, t, :], in_=adT_r[:, t, :])

        mask_sb = singles.tile([P, NKT], F32, tag="mask")
        nc.scalar.dma_start(out=mask_sb[:], in_=mask01[:])
        ones_sb = singles.tile([P, DH], F16, tag="ones")
        nc.vector.memset(ones_sb[:], 1.0)

        v_sb, wv_sb = load_stage(vT, nc.sync), load_w(wvT, nc.scalar)
        wo_sb = singles.tile([P, 4, DM], F16, tag="wo")
        nc.sync.dma_start(out=wo_sb[:], in_=woT.rearrange("(t p) d -> p t d", p=P))

        # Qt / Kt: out[m=dm-tile, n=l-chunk] = sum_din w?T[din, dm] * xT[din, l]
        for x_sb, w_sb, dst in ((q_sb, wq_sb, qt_sb), (k_sb, wk_sb, kt_sb)):
            for mt in range(4):
                for lc in range(NQC):
                    ps = pvwo.tile([P, QC], F32, tag="pvwo")
                    for kt2 in range(2):
                        nc.tensor.matmul(
                            ps[:],
                            w_sb[:, kt2, mt * P : (mt + 1) * P],
                            x_sb[:, kt2, lc * QC : (lc + 1) * QC],
                            start=(kt2 == 0),
                            stop=(kt2 == 1),
                        )
                    nc.vector.tensor_copy(
                        out=dst[:, mt, lc * QC : (lc + 1) * QC], in_=ps[:]
                    )

        # V: out[m=l-tile, n=dm] = sum_din vT[din, l] * wvT[din, dm]; mask rows
        for lt in range(NKT):
            ps = pvwo.tile([P, DM], F32, tag="pvwo")
            for kt2 in range(2):
                nc.tensor.matmul(
                    ps[:],
                    v_sb[:, kt2, lt * P : (lt + 1) * P],
                    wv_sb[:, kt2, :],
                    start=(kt2 == 0),
                    stop=(kt2 == 1),
                )
            nc.vector.tensor_scalar_mul(
                out=vx_sb[:, lt, :, 0:DH],
                in0=ps.rearrange("p (h d) -> p h d", h=H),
                scalar1=mask_sb[:, lt : lt + 1],
            )
            # mask column (softmax denominator counts only unmasked keys)
            nc.vector.tensor_copy(
                out=vx_sb[:, lt, :, DH : DH + 1],
                in_=mask_sb[:, lt : lt + 1, None].to_broadcast((P, H, 1)),
            )

        proj_ctx.close()

        # ---- phase 2: attention ----
        for qc in range(NQC):
            qs = slice(qc * QC, (qc + 1) * QC)
            for h in range(H):
                hb = (h % 2) * DH  # partition base of head h inside its dm-tile
                ht = h // 2
                ex = exps.tile([P, NKT, QC], F16, tag="ex")
                for ktp in range(NKT // 2):  # pairs of k-tiles share a psum
                    sp = spsum.tile([P, 2 * QC], F32, tag="sp")
                    # both scores first, then both ident pairs: consecutive
                    # instructions never share weights, so each LDWEIGHTS
                    # hides under the previous matmul's execution
                    for i in range(2):
                        kt = 2 * ktp + i
                        nc.tensor.matmul(
                            sp[:, i * QC : (i + 1) * QC],
                            kt_sb[hb : hb + DH, ht, kt * P : (kt + 1) * P],
                            qt_sb[hb : hb + DH, ht, qs],
                            start=True,
                            stop=False,
                        )
                    for i in range(2):
                        kt = 2 * ktp + i
                        nc.tensor.matmul(
                            sp[0:DH, i * QC : (i + 1) * QC],
                            idents[:, h, :], ad_sb[:, 2 * kt, qs],
                            start=False, stop=True, skip_group_check=True,
                        )
                        nc.tensor.matmul(
                            sp[DH:P, i * QC : (i + 1) * QC],
                            idents[:, h, :], ad_sb[:, 2 * kt + 1, qs],
                            start=False, stop=True, skip_group_check=True,
                        )
                    nc.scalar.activation(
                        out=ex[:, 2 * ktp : 2 * ktp + 2, :].rearrange(
                            "p a b -> p (a b)"
                        ),
                        in_=sp[:],
                        func=mybir.ActivationFunctionType.Exp,
                    )
                # PV with appended mask column -> row 64 = softmax denominator
                pv = pvwo.tile([P, QC], F32, tag="pvwo")
                for kt in range(NKT):
                    nc.tensor.matmul(
                        pv[0 : DH + 1, :],
                        vx_sb[:, kt, h, :],
                        ex[:, kt, :],
                        start=(kt == 0),
                        stop=(kt == NKT - 1),
                    )
                # normalize: shift denom row to partition 0, fast recip, fp16
                # cast, K=1 ones-matmul broadcast across 64 partitions
                den = small.tile([1, QC], F32, tag="den")
                nc.vector.tensor_copy(out=den[:], in_=pv[DH : DH + 1, :])
                rec = small.tile([1, QC], F32, tag="rec")
                nc.vector.reciprocal_approx_fast(out=rec[:], in_=den[:])
                rec16 = small.tile([1, QC], F16, tag="rec16")
                nc.vector.tensor_copy(out=rec16[:], in_=rec[:])
                bps = bcp.tile([DH, QC], F32, tag="bps")
                nc.tensor.matmul(
                    bps[:],
                    ones_sb[0:1, :],
                    rec16[:],
                    start=True,
                    stop=True,
                )
                pvs = small.tile([DH, QC], F32, tag="bc")
                nc.vector.tensor_copy(out=pvs[:], in_=pv[0:DH, :])
                # inputs share base 0; output base may differ (odd heads land
                # on partitions 64:128 directly)
                nc.vector.tensor_mul(
                    out=attnT_sb[hb : hb + DH, ht, :], in0=pvs[:], in1=bps[:]
                )

            # output projection for this q-chunk of rows
            for lt in range(QC // P):
                ws = pvwo.tile([P, DM], F32, tag="pvwo")
                for kt4 in range(4):
                    nc.tensor.matmul(
                        ws[:],
                        attnT_sb[:, kt4, lt * P : (lt + 1) * P],
                        wo_sb[:, kt4, :],
                        start=(kt4 == 0),
                        stop=(kt4 == 3),
                    )
                ost = small.tile([P, DM], F16, tag="ost")
                nc.scalar.copy(out=ost[:], in_=ws[:])
                nc.sync.dma_start(
                    out=out[qc * QC + lt * P : qc * QC + (lt + 1) * P, :], in_=ost[:]
                )


def build_nc():
    from concourse import bacc

    nc = bacc.Bacc("TRN2", target_bir_lowering=False, debug=False)
    with tile.TileContext(nc) as tc:
        _emit(tc)
    nc.compile()
    return nc


_NC = None


def _get_nc():
    global _NC
    if _NC is None:
        _NC = build_nc()
    return _NC


def make_in_maps(queries, keys, values, attention_mask, adjacency_matrix,
                 distance_matrix, W_q, W_k, W_v, W_o, lambda_a, lambda_d):
    f = np.float32
    h16 = np.float16
    c = np.ascontiguousarray
    wqT = c((W_q.astype(f) * f(0.125)).T).astype(h16)
    wkT = c(W_k.astype(f).T).astype(h16)
    wvT = c(W_v.astype(f).T).astype(h16)
    woT = c(W_o.astype(f).T).astype(h16)
    identsc = np.zeros((P, H, DH), dtype=f)
    rr = np.arange(DH)
    for h in range(H):
        identsc[rr, h, rr] = lambda_a[h]
        identsc[rr + DH, h, rr] = lambda_d[h]
    identsc = identsc.reshape(P, H * DH).astype(h16)
    in_maps = []
    for b in range(B):
        in_maps.append({
            "qT": c(queries[b].astype(f).T).astype(h16),
            "kT": c(keys[b].astype(f).T).astype(h16),
            "vT": c(values[b].astype(f).T).astype(h16),
            "wqT": wqT, "wkT": wkT, "wvT": wvT, "woT": woT,
            "adT": np.concatenate(
                [c(adjacency_matrix[b].astype(f).T).reshape(2 * NKT, DH, L),
                 c(distance_matrix[b].astype(f).T).reshape(2 * NKT, DH, L)],
                axis=1).reshape(2 * L, L).astype(h16),
            "mask01": c((attention_mask[b] > 0).astype(f).reshape(NKT, P).T),
            "identsc": identsc,
        })
    return in_maps


def kernel(queries, keys, values, attention_mask, adjacency_matrix,
           distance_matrix, W_q, W_k, W_v, W_o, lambda_a, lambda_d, **kw):
    nc = _get_nc()
    in_maps = make_in_maps(queries, keys, values, attention_mask,
                           adjacency_matrix, distance_matrix,
                           W_q, W_k, W_v, W_o, lambda_a, lambda_d)
    res = run_bass_kernel_spmd(nc, in_maps, list(range(B)), **kw)
    outs = np.stack([res.results[i]["out"] for i in range(B)]).astype(np.float32)
    return outs


# revision 19
# speedup vs baseline: 1.0063x; 1.0063x over previous
"""MSRSA multi-head attention kernel for 8 Trainium2 NeuronCores.

Strategy: data-parallel over batch (B=8 -> 1 batch element per core).
Per core, for its batch element b:
  Qt = (W_q/8) @ queries^T        [512,1024]  (scale 1/8 folded into W_q)
  Kt = W_k @ keys^T               [512,1024]
  V  = values @ W_v^T             [1024,512]  (rows masked by attention_mask)
  per head h, scores are computed TRANSPOSED: S_T[k,q]:
     S_T = sum_d Kt[d,k]*Qt[d,q] + la[h]*A^T[k,q] + ld[h]*D^T[k,q]
  (A/D bias injected by scaled-identity matmuls accumulating into PSUM)
  expS = exp(S_T) on ScalarE (PSUM -> SBUF evacuation is the exp)
  attnT_h[d,q] (+ denominator row) = sum_k V_ext[k, d|mask] * expS[k,q]
  (mask column of V_ext -> row 64 of PV output = softmax denominator)
  normalize: denom row copied to partition 0 (copies may retarget the output
  base), reciprocal_approx_fast at base 0, fp16 cast, K=1 ones-matmul
  broadcast; the multiply writes even heads at partitions 0:64 and odd heads
  at 64:128 directly.
  out = attnT contracted with W_o^T   [1024, 512] (fp16, host upcasts)

On TRN2 a matmul costs ~N output columns at 1 col/cycle regardless of dtype
or contraction size, so the kernel minimizes matmul COUNT and keeps weight
loads small (fp16 64-col ident tiles) so LDWEIGHTS hides under execution.
"""

import contextlib

import numpy as np

import concourse.bass as bass
import concourse.mybir as mybir
import concourse.tile as tile
from concourse.bass_utils import run_bass_kernel_spmd

B, L, DIN, DM, H = 8, 1024, 256, 512, 8
DH = DM // H  # 64
P = 128
NKT = L // P          # 8 k-tiles
NQC = 2               # q chunks
QC = L // NQC         # 512
F32 = mybir.dt.float32
F16 = mybir.dt.float16


def _emit(tc):
    nc = tc.nc

    def dram(name, shape, dtype=F16, kind="ExternalInput"):
        return nc.dram_tensor(name, shape, dtype, kind=kind).ap()

    qT = dram("qT", [DIN, L])
    kT = dram("kT", [DIN, L])
    vT = dram("vT", [DIN, L])
    wqT = dram("wqT", [DIN, DM])
    wkT = dram("wkT", [DIN, DM])
    wvT = dram("wvT", [DIN, DM])
    woT = dram("woT", [DM, DM])
    adT = dram("adT", [2 * L, L])  # A^T/D^T interleaved in 64-row blocks
    identsc = dram("identsc", [P, H * DH])
    mask01 = dram("mask01", [P, NKT], F32)
    out = dram("out", [L, DM], F16, kind="ExternalOutput")

    with contextlib.ExitStack() as ctx:
        singles = ctx.enter_context(tc.tile_pool(name="singles", bufs=1))
        big = ctx.enter_context(tc.tile_pool(name="big", bufs=1))
        exps = ctx.enter_context(tc.tile_pool(name="exps", bufs=3))
        small = ctx.enter_context(tc.tile_pool(name="small", bufs=2))
        spsum = ctx.enter_context(tc.tile_pool(name="spsum", bufs=2, space="PSUM"))
        pvwo = ctx.enter_context(tc.tile_pool(name="pvwo", bufs=3, space="PSUM"))
        bcp = ctx.enter_context(tc.tile_pool(name="bcp", bufs=1, space="PSUM"))

        # ---- big SBUF-resident tensors ----
        ad_sb = big.tile([P, 2 * NKT, L], F16, tag="ad")  # A^T|D^T 64-row blocks
        qt_sb = big.tile([P, 4, L], F16, tag="qt")       # [p,t,l] = Qt[t*128+p, l]
        kt_sb = big.tile([P, 4, L], F16, tag="kt")
        vx_sb = big.tile([P, NKT, H, DH + 1], F16, tag="vx")  # V + mask column
        attnT_sb = big.tile([P, 4, QC], F16, tag="attnT")     # per q-chunk

        adT_r = adT.rearrange("(t p) q -> p t q", p=P)

        # ---- phase 1: projections (pools scoped so SBUF is reclaimed) ----
        proj_ctx = contextlib.ExitStack()
        stage = proj_ctx.enter_context(tc.tile_pool(name="stage", bufs=3))
        wpool = proj_ctx.enter_context(tc.tile_pool(name="wpool", bufs=3))

        def load_stage(src, eng):
            t = stage.tile([P, 2, L], F16, tag="stage")
            r = src.rearrange("(t p) l -> p t l", p=P)
            for i in range(2):  # per-half DMAs so the first matmul starts early
                eng.dma_start(out=t[:, i, :], in_=r[:, i, :])
            return t

        def load_w(src, eng):
            t = wpool.tile([P, 2, DM], F16, tag="w")
            r = src.rearrange("(t p) d -> p t d", p=P)
            for i in range(2):
                eng.dma_start(out=t[:, i, :], in_=r[:, i, :])
            return t

        # DMA issue order = dependency order: Q/K paths gate the first
        # matmuls, idents+ad gate the first bias matmul, V/Wo come later.
        # Issue across both HWDGE engines (sync + scalar) so descriptor
        # generation is not serialized at the head of the kernel.
        q_sb, wq_sb = load_stage(qT, nc.sync), load_w(wqT, nc.scalar)
        k_sb, wk_sb = load_stage(kT, nc.sync), load_w(wkT, nc.scalar)

        idents = singles.tile([P, H, DH], F16, tag="idents")
        nc.scalar.dma_start(
            out=idents[:], in_=identsc.rearrange("p (j m) -> p j m", m=DH)
        )
        for t in range(2 * NKT):
            eng = nc.sync if t % 2 == 0 else nc.scalar
            eng.dma_start(out=ad_sb[:, t, :], in_=adT_r[:, t, :])

        mask_sb = singles.tile([P, NKT], F32, tag="mask")
        nc.scalar.dma_start(out=mask_sb[:], in_=mask01[:])
        ones_sb = singles.tile([P, DH], F16, tag="ones")
        nc.vector.memset(ones_sb[:], 1.0)

        v_sb, wv_sb = load_stage(vT, nc.sync), load_w(wvT, nc.scalar)
        wo_sb = singles.tile([P, 4, DM], F16, tag="wo")
        nc.sync.dma_start(out=wo_sb[:], in_=woT.rearrange("(t p) d -> p t d", p=P))

        # Qt / Kt: out[m=dm-tile, n=l-chunk] = sum_din w?T[din, dm] * xT[din, l]
        for x_sb, w_sb, dst in ((q_sb, wq_sb, qt_sb), (k_sb, wk_sb, kt_sb)):
            for mt in range(4):
                for lc in range(NQC):
                    ps = pvwo.tile([P, QC], F32, tag="pvwo")
                    for kt2 in range(2):
                        nc.tensor.matmul(
                            ps[:],
                            w_sb[:, kt2, mt * P : (mt + 1) * P],
                            x_sb[:, kt2, lc * QC : (lc + 1) * QC],
                            start=(kt2 == 0),
                            stop=(kt2 == 1),
                        )
                    nc.vector.tensor_copy(
                        out=dst[:, mt, lc * QC : (lc + 1) * QC], in_=ps[:]
                    )

        # V: out[m=l-tile, n=dm] = sum_din vT[din, l] * wvT[din, dm]; mask rows
        for lt in range(NKT):
            ps = pvwo.tile([P, DM], F32, tag="pvwo")
            for kt2 in range(2):
                nc.tensor.matmul(
                    ps[:],
                    v_sb[:, kt2, lt * P : (lt + 1) * P],
                    wv_sb[:, kt2, :],
                    start=(kt2 == 0),
                    stop=(kt2 == 1),
                )
            nc.vector.tensor_scalar_mul(
                out=vx_sb[:, lt, :, 0:DH],
                in0=ps.rearrange("p (h d) -> p h d", h=H),
                scalar1=mask_sb[:, lt : lt + 1],
            )
            # mask column (softmax denominator counts only unmasked keys)
            nc.vector.tensor_copy(
                out=vx_sb[:, lt, :, DH : DH + 1],
                in_=mask_sb[:, lt : lt + 1, None].to_broadcast((P, H, 1)),
            )

        proj_ctx.close()

        # ---- phase 2: attention ----
        for qc in range(NQC):
            qs = slice(qc * QC, (qc + 1) * QC)
            for h in range(H):
                hb = (h % 2) * DH  # partition base of head h inside its dm-tile
                ht = h // 2
                ex = exps.tile([P, NKT, QC], F16, tag="ex")
                for ktp in range(NKT // 2):  # pairs of k-tiles share a psum
                    sp = spsum.tile([P, 2 * QC], F32, tag="sp")
                    # both scores first, then both ident pairs: consecutive
                    # instructions never share weights, so each LDWEIGHTS
                    # hides under the previous matmul's execution
                    for i in range(2):
                        kt = 2 * ktp + i
                        nc.tensor.matmul(
                            sp[:, i * QC : (i + 1) * QC],
                            kt_sb[hb : hb + DH, ht, kt * P : (kt + 1) * P],
                            qt_sb[hb : hb + DH, ht, qs],
                            start=True,
                            stop=False,
                        )
                    for i in range(2):
                        kt = 2 * ktp + i
                        nc.tensor.matmul(
                            sp[0:DH, i * QC : (i + 1) * QC],
                            idents[:, h, :], ad_sb[:, 2 * kt, qs],
                            start=False, stop=True, skip_group_check=True,
                        )
                        nc.tensor.matmul(
                            sp[DH:P, i * QC : (i + 1) * QC],
                            idents[:, h, :], ad_sb[:, 2 * kt + 1, qs],
                            start=False, stop=True, skip_group_check=True,
                        )
                    nc.scalar.activation(
                        out=ex[:, 2 * ktp : 2 * ktp + 2, :].rearrange(
                            "p a b -> p (a b)"
                        ),
                        in_=sp[:],
                        func=mybir.ActivationFunctionType.Exp,
                    )
                # PV with appended mask column -> row 64 = softmax denominator
                pv = pvwo.tile([P, QC], F32, tag="pvwo")
                for kt in range(NKT):
                    nc.tensor.matmul(
                        pv[0 : DH + 1, :],
                        vx_sb[:, kt, h, :],
                        ex[:, kt, :],
                        start=(kt == 0),
                        stop=(kt == NKT - 1),
                    )
                # normalize: shift denom row to partition 0, fast recip, fp16
                # cast, K=1 ones-matmul broadcast across 64 partitions
                den = small.tile([1, QC], F32, tag="den")
                nc.vector.tensor_copy(out=den[:], in_=pv[DH : DH + 1, :])
                rec = small.tile([1, QC], F32, tag="rec")
                nc.vector.reciprocal_approx_fast(out=rec[:], in_=den[:])
                rec16 = small.tile([1, QC], F16, tag="rec16")
                nc.vector.tensor_copy(out=rec16[:], in_=rec[:])
                bps = bcp.tile([DH, QC], F32, tag="bps")
                nc.tensor.matmul(
                    bps[:],
                    ones_sb[0:1, :],
                    rec16[:],
                    start=True,
                    stop=True,
                )
                pvs = small.tile([DH, QC], F32, tag="bc")
                nc.vector.tensor_copy(out=pvs[:], in_=pv[0:DH, :])
                # inputs share base 0; output base may differ (odd heads land
                # on partitions 64:128 directly)
                nc.vector.tensor_mul(
                    out=attnT_sb[hb : hb + DH, ht, :], in0=pvs[:], in1=bps[:]
                )

            # output projection for this q-chunk of rows
            for lt in range(QC // P):
                ws = pvwo.tile([P, DM], F32, tag="pvwo")
                for kt4 in range(4):
                    nc.tensor.matmul(
                        ws[:],
                        attnT_sb[:, kt4, lt * P : (lt + 1) * P],
                        wo_sb[:, kt4, :],
                        start=(kt4 == 0),
                        stop=(kt4 == 3),
                    )
                ost = small.tile([P, DM], F16, tag="ost")
                nc.scalar.copy(out=ost[:], in_=ws[:])
                nc.sync.dma_start(
                    out=out[qc * QC + lt * P : qc * QC + (lt + 1) * P, :], in_=ost[:]
                )


def build_nc():
    from concourse import bacc

    nc = bacc.Bacc("TRN2", target_bir_lowering=False, debug=False)
    with tile.TileContext(nc) as tc:
        _emit(tc)
    nc.compile()
    return nc


_NC = None


def _get_nc():
    global _NC
    if _NC is None:
        _NC = build_nc()
    return _NC


def make_in_maps(queries, keys, values, attention_mask, adjacency_matrix,
                 distance_matrix, W_q, W_k, W_v, W_o, lambda_a, lambda_d):
    f = np.float32
    h16 = np.float16
    c = np.ascontiguousarray
    wqT = c((W_q.astype(f) * f(0.125)).T).astype(h16)
    wkT = c(W_k.astype(f).T).astype(h16)
    wvT = c(W_v.astype(f).T).astype(h16)
    woT = c(W_o.astype(f).T).astype(h16)
    identsc = np.zeros((P, H, DH), dtype=f)
    rr = np.arange(DH)
    for h in range(H):
        identsc[rr, h, rr] = lambda_a[h]
        identsc[rr + DH, h, rr] = lambda_d[h]
    identsc = identsc.reshape(P, H * DH).astype(h16)
    in_maps = []
    for b in range(B):
        in_maps.append({
            "qT": c(queries[b].astype(f).T).astype(h16),
            "kT": c(keys[b].astype(f).T).astype(h16),
            "vT": c(values[b].astype(f).T).astype(h16),
            "wqT": wqT, "wkT": wkT, "wvT": wvT, "woT": woT,
            "adT": np.concatenate(
                [c(adjacency_matrix[b].astype(f).T).reshape(2 * NKT, DH, L),
                 c(distance_matrix[b].astype(f).T).reshape(2 * NKT, DH, L)],
                axis=1).reshape(2 * L, L).astype(h16),
            "mask01": c((attention_mask[b] > 0).astype(f).reshape(NKT, P).T),
            "identsc": identsc,
        })
    return in_maps


def kernel(queries, keys, values, attention_mask, adjacency_matrix,
           distance_matrix, W_q, W_k, W_v, W_o, lambda_a, lambda_d, **kw):
    nc = _get_nc()
    in_maps = make_in_maps(queries, keys, values, attention_mask,
                           adjacency_matrix, distance_matrix,
                           W_q, W_k, W_v, W_o, lambda_a, lambda_d)
    res = run_bass_kernel_spmd(nc, in_maps, list(range(B)), **kw)
    outs = np.stack([res.results[i]["out"] for i in range(B)]).astype(np.float32)
    return outs


# revision 20
# speedup vs baseline: 1.0083x; 1.0020x over previous
"""MSRSA multi-head attention kernel for 8 Trainium2 NeuronCores.

Strategy: data-parallel over batch (B=8 -> 1 batch element per core).
Per core, for its batch element b:
  Qt = (W_q/8) @ queries^T        [512,1024]  (scale 1/8 folded into W_q)
  Kt = W_k @ keys^T               [512,1024]
  V  = values @ W_v^T             [1024,512]  (rows masked by attention_mask)
  per head h, scores are computed TRANSPOSED: S_T[k,q]:
     S_T = sum_d Kt[d,k]*Qt[d,q] + la[h]*A^T[k,q] + ld[h]*D^T[k,q]
  (A/D bias injected by scaled-identity matmuls accumulating into PSUM)
  expS = exp(S_T) on ScalarE (PSUM -> SBUF evacuation is the exp)
  attnT_h[d,q] (+ denominator row) = sum_k V_ext[k, d|mask] * expS[k,q]
  (mask column of V_ext -> row 64 of PV output = softmax denominator)
  normalize: denom row copied to partition 0 (copies may retarget the output
  base), reciprocal_approx_fast at base 0, fp16 cast, K=1 ones-matmul
  broadcast; the multiply writes even heads at partitions 0:64 and odd heads
  at 64:128 directly.
  out = attnT contracted with W_o^T   [1024, 512] (fp16, host upcasts)

On TRN2 a matmul costs ~N output columns at 1 col/cycle regardless of dtype
or contraction size, so the kernel minimizes matmul COUNT and keeps weight
loads small (fp16 64-col ident tiles) so LDWEIGHTS hides under execution.
"""

import contextlib

import numpy as np

import concourse.bass as bass
import concourse.mybir as mybir
import concourse.tile as tile
from concourse.bass_utils import run_bass_kernel_spmd

B, L, DIN, DM, H = 8, 1024, 256, 512, 8
DH = DM // H  # 64
P = 128
NKT = L // P          # 8 k-tiles
NQC = 2               # q chunks
QC = L // NQC         # 512
F32 = mybir.dt.float32
F16 = mybir.dt.float16


def _emit(tc):
    nc = tc.nc

    def dram(name, shape, dtype=F16, kind="ExternalInput"):
        return nc.dram_tensor(name, shape, dtype, kind=kind).ap()

    qT = dram("qT", [DIN, L])
    kT = dram("kT", [DIN, L])
    vT = dram("vT", [DIN, L])
    wqT = dram("wqT", [DIN, DM])
    wkT = dram("wkT", [DIN, DM])
    wvT = dram("wvT", [DIN, DM])
    woT = dram("woT", [DM, DM])
    adT = dram("adT", [2 * L, L])  # A^T/D^T interleaved in 64-row blocks
    identsc = dram("identsc", [P, H * DH])
    mask01 = dram("mask01", [P, NKT], F32)
    out = dram("out", [L, DM], F16, kind="ExternalOutput")

    with contextlib.ExitStack() as ctx:
        singles = ctx.enter_context(tc.tile_pool(name="singles", bufs=1))
        big = ctx.enter_context(tc.tile_pool(name="big", bufs=1))
        exps = ctx.enter_context(tc.tile_pool(name="exps", bufs=3))
        small = ctx.enter_context(tc.tile_pool(name="small", bufs=2))
        spsum = ctx.enter_context(tc.tile_pool(name="spsum", bufs=2, space="PSUM"))
        pvwo = ctx.enter_context(tc.tile_pool(name="pvwo", bufs=3, space="PSUM"))
        bcp = ctx.enter_context(tc.tile_pool(name="bcp", bufs=1, space="PSUM"))

        # ---- big SBUF-resident tensors ----
        ad_sb = big.tile([P, 2 * NKT, L], F16, tag="ad")  # A^T|D^T 64-row blocks
        qt_sb = big.tile([P, 4, L], F16, tag="qt")       # [p,t,l] = Qt[t*128+p, l]
        kt_sb = big.tile([P, 4, L], F16, tag="kt")
        vx_sb = big.tile([P, NKT, H, DH + 1], F16, tag="vx")  # V + mask column
        attnT_sb = big.tile([P, 4, QC], F16, tag="attnT")     # per q-chunk

        adT_r = adT.rearrange("(t p) q -> p t q", p=P)

        # ---- phase 1: projections (pools scoped so SBUF is reclaimed) ----
        proj_ctx = contextlib.ExitStack()
        stage = proj_ctx.enter_context(tc.tile_pool(name="stage", bufs=3))
        wpool = proj_ctx.enter_context(tc.tile_pool(name="wpool", bufs=3))

        def load_stage(src, eng):
            t = stage.tile([P, 2, L], F16, tag="stage")
            r = src.rearrange("(t p) l -> p t l", p=P)
            for i in range(2):  # per-half DMAs so the first matmul starts early
                eng.dma_start(out=t[:, i, :], in_=r[:, i, :])
            return t

        def load_w(src, eng):
            t = wpool.tile([P, 2, DM], F16, tag="w")
            r = src.rearrange("(t p) d -> p t d", p=P)
            for i in range(2):
                eng.dma_start(out=t[:, i, :], in_=r[:, i, :])
            return t

        # DMA issue order = dependency order: Q/K paths gate the first
        # matmuls, idents+ad gate the first bias matmul, V/Wo come later.
        # Issue across both HWDGE engines (sync + scalar) so descriptor
        # generation is not serialized at the head of the kernel.
        q_sb, wq_sb = load_stage(qT, nc.sync), load_w(wqT, nc.scalar)
        k_sb, wk_sb = load_stage(kT, nc.sync), load_w(wkT, nc.scalar)

        idents = singles.tile([P, H, DH], F16, tag="idents")
        nc.scalar.dma_start(
            out=idents[:], in_=identsc.rearrange("p (j m) -> p j m", m=DH)
        )
        for t in range(2 * NKT):
            eng = nc.sync if t % 2 == 0 else nc.scalar
            eng.dma_start(out=ad_sb[:, t, :], in_=adT_r[:, t, :])

        mask_sb = singles.tile([P, NKT], F32, tag="mask")
        nc.scalar.dma_start(out=mask_sb[:], in_=mask01[:])
        ones_sb = singles.tile([P, DH], F16, tag="ones")
        nc.vector.memset(ones_sb[:], 1.0)

        v_sb, wv_sb = load_stage(vT, nc.sync), load_w(wvT, nc.scalar)
        wo_sb = singles.tile([P, 4, DM], F16, tag="wo")
        nc.sync.dma_start(out=wo_sb[:], in_=woT.rearrange("(t p) d -> p t d", p=P))

        # Qt / Kt: out[m=dm-tile, n=l-chunk] = sum_din w?T[din, dm] * xT[din, l]
        for x_sb, w_sb, dst in ((q_sb, wq_sb, qt_sb), (k_sb, wk_sb, kt_sb)):
            for mt in range(4):
                for lc in range(NQC):
                    ps = pvwo.tile([P, QC], F32, tag="pvwo")
                    for kt2 in range(2):
                        nc.tensor.matmul(
                            ps[:],
                            w_sb[:, kt2, mt * P : (mt + 1) * P],
                            x_sb[:, kt2, lc * QC : (lc + 1) * QC],
                            start=(kt2 == 0),
                            stop=(kt2 == 1),
                        )
                    nc.vector.tensor_copy(
                        out=dst[:, mt, lc * QC : (lc + 1) * QC], in_=ps[:]
                    )

        # V: out[m=l-tile, n=dm] = sum_din vT[din, l] * wvT[din, dm]; mask rows
        for lt in range(NKT):
            ps = pvwo.tile([P, DM], F32, tag="pvwo")
            for kt2 in range(2):
                nc.tensor.matmul(
                    ps[:],
                    v_sb[:, kt2, lt * P : (lt + 1) * P],
                    wv_sb[:, kt2, :],
                    start=(kt2 == 0),
                    stop=(kt2 == 1),
                )
            nc.vector.tensor_scalar_mul(
                out=vx_sb[:, lt, :, 0:DH],
                in0=ps.rearrange("p (h d) -> p h d", h=H),
                scalar1=mask_sb[:, lt : lt + 1],
            )
            # mask column (softmax denominator counts only unmasked keys)
            nc.vector.tensor_copy(
                out=vx_sb[:, lt, :, DH : DH + 1],
                in_=mask_sb[:, lt : lt + 1, None].to_broadcast((P, H, 1)),
            )

        proj_ctx.close()

        # ---- phase 2: attention ----
        for qc in range(NQC):
            qs = slice(qc * QC, (qc + 1) * QC)
            for h in range(H):
                hb = (h % 2) * DH  # partition base of head h inside its dm-tile
                ht = h // 2
                ex = exps.tile([P, NKT, QC], F16, tag="ex")
                for ktp in range(NKT // 2):  # pairs of k-tiles share a psum
                    sp = spsum.tile([P, 2 * QC], F32, tag="sp")
                    # ident pairs start (reset) the PSUM regions — the two
                    # 64-row matmuls overlap in PE quadrants so the reset
                    # cost hides; scores then accumulate and stop
                    for i in range(2):
                        kt = 2 * ktp + i
                        nc.tensor.matmul(
                            sp[0:DH, i * QC : (i + 1) * QC],
                            idents[:, h, :], ad_sb[:, 2 * kt, qs],
                            start=True, stop=False, skip_group_check=True,
                        )
                        nc.tensor.matmul(
                            sp[DH:P, i * QC : (i + 1) * QC],
                            idents[:, h, :], ad_sb[:, 2 * kt + 1, qs],
                            start=True, stop=False, skip_group_check=True,
                        )
                    for i in range(2):
                        kt = 2 * ktp + i
                        nc.tensor.matmul(
                            sp[:, i * QC : (i + 1) * QC],
                            kt_sb[hb : hb + DH, ht, kt * P : (kt + 1) * P],
                            qt_sb[hb : hb + DH, ht, qs],
                            start=False,
                            stop=True,
                            skip_group_check=True,
                        )
                    nc.scalar.activation(
                        out=ex[:, 2 * ktp : 2 * ktp + 2, :].rearrange(
                            "p a b -> p (a b)"
                        ),
                        in_=sp[:],
                        func=mybir.ActivationFunctionType.Exp,
                    )
                # PV with appended mask column -> row 64 = softmax denominator
                pv = pvwo.tile([P, QC], F32, tag="pvwo")
                for kt in range(NKT):
                    nc.tensor.matmul(
                        pv[0 : DH + 1, :],
                        vx_sb[:, kt, h, :],
                        ex[:, kt, :],
                        start=(kt == 0),
                        stop=(kt == NKT - 1),
                    )
                # normalize: shift denom row to partition 0, fast recip, fp16
                # cast, K=1 ones-matmul broadcast across 64 partitions
                den = small.tile([1, QC], F32, tag="den")
                nc.vector.tensor_copy(out=den[:], in_=pv[DH : DH + 1, :])
                rec = small.tile([1, QC], F32, tag="rec")
                nc.vector.reciprocal_approx_fast(out=rec[:], in_=den[:])
                rec16 = small.tile([1, QC], F16, tag="rec16")
                nc.vector.tensor_copy(out=rec16[:], in_=rec[:])
                bps = bcp.tile([DH, QC], F32, tag="bps")
                nc.tensor.matmul(
                    bps[:],
                    ones_sb[0:1, :],
                    rec16[:],
                    start=True,
                    stop=True,
                )
                pvs = small.tile([DH, QC], F32, tag="bc")
                nc.vector.tensor_copy(out=pvs[:], in_=pv[0:DH, :])
                # inputs share base 0; output base may differ (odd heads land
                # on partitions 64:128 directly)
                nc.vector.tensor_mul(
                    out=attnT_sb[hb : hb + DH, ht, :], in0=pvs[:], in1=bps[:]
                )

            # output projection for this q-chunk of rows
            for lt in range(QC // P):
                ws = pvwo.tile([P, DM], F32, tag="pvwo")
                for kt4 in range(4):
                    nc.tensor.matmul(
                        ws[:],
                        attnT_sb[:, kt4, lt * P : (lt + 1) * P],
                        wo_sb[:, kt4, :],
                        start=(kt4 == 0),
                        stop=(kt4 == 3),
                    )
                ost = small.tile([P, DM], F16, tag="ost")
                nc.scalar.copy(out=ost[:], in_=ws[:])
                nc.sync.dma_start(
                    out=out[qc * QC + lt * P : qc * QC + (lt + 1) * P, :], in_=ost[:]
                )


def build_nc():
    from concourse import bacc

    nc = bacc.Bacc("TRN2", target_bir_lowering=False, debug=False)
    with tile.TileContext(nc) as tc:
        _emit(tc)
    nc.compile()
    return nc


_NC = None


def _get_nc():
    global _NC
    if _NC is None:
        _NC = build_nc()
    return _NC


def make_in_maps(queries, keys, values, attention_mask, adjacency_matrix,
                 distance_matrix, W_q, W_k, W_v, W_o, lambda_a, lambda_d):
    f = np.float32
    h16 = np.float16
    c = np.ascontiguousarray
    wqT = c((W_q.astype(f) * f(0.125)).T).astype(h16)
    wkT = c(W_k.astype(f).T).astype(h16)
    wvT = c(W_v.astype(f).T).astype(h16)
    woT = c(W_o.astype(f).T).astype(h16)
    identsc = np.zeros((P, H, DH), dtype=f)
    rr = np.arange(DH)
    for h in range(H):
        identsc[rr, h, rr] = lambda_a[h]
        identsc[rr + DH, h, rr] = lambda_d[h]
    identsc = identsc.reshape(P, H * DH).astype(h16)
    in_maps = []
    for b in range(B):
        in_maps.append({
            "qT": c(queries[b].astype(f).T).astype(h16),
            "kT": c(keys[b].astype(f).T).astype(h16),
            "vT": c(values[b].astype(f).T).astype(h16),
            "wqT": wqT, "wkT": wkT, "wvT": wvT, "woT": woT,
            "adT": np.concatenate(
                [c(adjacency_matrix[b].astype(f).T).reshape(2 * NKT, DH, L),
                 c(distance_matrix[b].astype(f).T).reshape(2 * NKT, DH, L)],
                axis=1).reshape(2 * L, L).astype(h16),
            "mask01": c((attention_mask[b] > 0).astype(f).reshape(NKT, P).T),
            "identsc": identsc,
        })
    return in_maps


def kernel(queries, keys, values, attention_mask, adjacency_matrix,
           distance_matrix, W_q, W_k, W_v, W_o, lambda_a, lambda_d, **kw):
    nc = _get_nc()
    in_maps = make_in_maps(queries, keys, values, attention_mask,
                           adjacency_matrix, distance_matrix,
                           W_q, W_k, W_v, W_o, lambda_a, lambda_d)
    res = run_bass_kernel_spmd(nc, in_maps, list(range(B)), **kw)
    outs = np.stack([res.results[i]["out"] for i in range(B)]).astype(np.float32)
    return outs
